# revision 1
# baseline (speedup 1.0000x reference)
"""Single-head attention (nn_MultiHeadAttention) Trainium2 Bass kernel.

Full inputs: x [4, 2048, 1024], Wq/Wk/Wv/Wo [1024, 1024], biases [1024].
reference:  q = x @ Wq.T + bq ; k,v likewise
            scores = (q @ k.T) / sqrt(1024) ; attn = softmax(scores, -1)
            out = (attn @ v) @ Wo.T + bo

Sharding: 8 cores = 4 batches x 2 query-halves. Each core computes the
full K/V projection of its batch (duplicated across the pair) and
attention + output projection for its 1024 queries.

Host-side prep per core (b = c // 2, h = c % 2):
  xT = concat(x[b, h-half].T, x[b, other-half].T) -> [1024, 2048]
  (queries always occupy the first 1024 columns; the key order is a
   permutation, to which softmax attention is invariant)
  W*T = W*.T (so the contraction dim lands on SBUF partitions)

Per-core pipeline (matmuls in float32r = single-pass fp22 PE mode):
  V phase:  V[s,e]   = xT.T @ WvT (+bv)           -> resident SBUF
  K phase:  KT[e,s]  = WkT.T @ xT (+bk)           -> spilled to DRAM scratch
  Q phase:  QT[e,sq] = WqT.T @ xT[:, :1024] (+bq) -> resident
  scores:   uT[sk,sq] = exp((KT.T @ QT) / 32)     (no max-sub; |scores| < ~7)
            Z[1,sq] += ones.T @ uT                 (PE column-sum)
  Z:        PE-transpose 128-chunks of Z, reciprocal -> rZT[sq,1]
  ctx:      ctxT[e,sq] = V.T @ uT                  (V tiles stationary)
  out:      out[sq,f] = (ctxT.T @ WoT) * rZT + bo
"""

import numpy as np
from contextlib import ExitStack

import concourse.bass as bass
import concourse.bacc as bacc
import concourse.mybir as mybir
import concourse.tile as tile
from concourse import bass_utils
from concourse.masks import make_identity

F32 = mybir.dt.float32
F32R = mybir.dt.float32r
AF = mybir.ActivationFunctionType
ALU = mybir.AluOpType

B, S, D = 4, 2048, 1024
SQ = S // 2  # queries per core
N_CORES = 8


def build_nc(S=S, D=D, SQ=SQ):
    P = 128
    DT = D // P          # contraction tiles (8)
    ET = D // P          # output-dim tiles (8)
    NBW = min(512, D)    # free-dim block over D
    NB = D // NBW        # (2)
    SBW = min(512, S)    # free-dim block over S
    SKB = S // SBW       # (4)
    SKT = S // P         # key tiles (16)
    SQW = min(512, SQ)
    SQB = SQ // SQW      # (2)
    SQT = SQ // P        # query tiles (8)
    SCALE = 1.0 / float(np.sqrt(D))

    nc = bacc.Bacc("TRN2", target_bir_lowering=False, debug=False)

    xT = nc.dram_tensor("xT", [D, S], F32R, kind="ExternalInput")
    wqT = nc.dram_tensor("wqT", [D, D], F32R, kind="ExternalInput")
    wkT = nc.dram_tensor("wkT", [D, D], F32R, kind="ExternalInput")
    wvT = nc.dram_tensor("wvT", [D, D], F32R, kind="ExternalInput")
    woT = nc.dram_tensor("woT", [D, D], F32R, kind="ExternalInput")
    bqd = nc.dram_tensor("bq", [D], F32, kind="ExternalInput")
    bkd = nc.dram_tensor("bk", [D], F32, kind="ExternalInput")
    bvd = nc.dram_tensor("bv", [D], F32, kind="ExternalInput")
    bod = nc.dram_tensor("bo", [D], F32, kind="ExternalInput")
    outd = nc.dram_tensor("out", [SQ, D], F32, kind="ExternalOutput")

    def bcast_ap(handle):
        a = handle[:]
        return bass.AP(tensor=a.tensor, offset=a.offset, ap=[[0, P]] + list(a.ap))

    with tile.TileContext(nc) as tc, ExitStack() as top:
        singles = top.enter_context(tc.tile_pool(name="singles", bufs=1))
        dram = top.enter_context(tc.tile_pool(name="dram", bufs=1, space="DRAM"))
        psum_mm = top.enter_context(tc.tile_pool(name="psum_mm", bufs=5, space="PSUM"))
        psum_z = top.enter_context(tc.tile_pool(name="psum_z", bufs=2, space="PSUM"))
        psum_tr = top.enter_context(tc.tile_pool(name="psum_tr", bufs=1, space="PSUM"))

        ktd = dram.tile([D, S], F32R, name="ktd", tag="ktd")


        # V resident for ctx; allocated first on the right stack (LIFO: wo,
        # ctx, qt pop before it).
        v_pool = tc.alloc_tile_pool(name="v", bufs=SKT, side="right")
        v_tiles = [v_pool.tile([P, D], F32R, name=f"v{i}", tag="v") for i in range(SKT)]

        # ---------------- Q / K / V phases (xT resident) ----------------
        # Phase order is chosen so the PE can start as soon as the query
        # half of xT (4MB) and the first wq column (0.5MB) land, with the
        # rest of xT / weight columns streaming behind compute.
        with tc.tile_pool(name="xt", bufs=2 * DT) as xt_pool:
            # two separate tiles per d-tile (query half / key-tail half) so
            # Q-phase matmuls depend only on the first 4MB of xT
            xr = xT[:].rearrange("(t p) s -> t p s", p=P)
            xta_tiles = []
            xtb_tiles = []
            for t in range(DT):
                xta = xt_pool.tile([P, SQ], F32R, name=f"xta{t}", tag="xt")
                nc.sync.dma_start(out=xta, in_=xr[t][:, 0:SQ])
                xta_tiles.append(xta)

            def xt_slice(d, lo, width):
                """Columns [lo, lo+width) of logical xT d-tile; never spans
                the SQ boundary by construction."""
                if lo < SQ:
                    return xta_tiles[d][:, lo:lo + width]
                return xtb_tiles[d][:, lo - SQ:lo - SQ + width]

            # wq prefetch (depth = wcol bufs) ahead of the bias setup so the
            # Q phase's first columns beat the bias DMAs to the sem lanes
            wc_pool = tc.alloc_tile_pool(name="wcol", bufs=2)

            def load_wcol(wt, et, nm):
                col = wc_pool.tile([P, DT, P], F32R, name=nm, tag="wc")
                nc.sync.dma_start(
                    out=col,
                    in_=wt[:, et * P:(et + 1) * P].rearrange("(t p) e -> p t e", p=P),
                )
                return col

            wq_next = [load_wcol(wqT, 0, "wq"), load_wcol(wqT, 1, "wq")]

            # constants + biases: emitted after the xta loads so those grab
            # the DMA semaphore lanes first (these are not start-critical)
            ones_f32 = singles.tile([P, 1], F32, name="ones_f32", tag="ones_f32")
            nc.vector.memset(ones_f32, 1.0)
            ones_col = singles.tile([P, 1], F32R, name="ones_col", tag="ones_col")
            nc.scalar.activation(out=ones_col, in_=ones_f32, func=AF.Copy)
            ident = singles.tile([P, P], F32, name="ident", tag="ident")
            make_identity(nc, ident)
            # per-partition bias layouts [p, t] = b[t*128 + p] (e on partitions)
            bq_pt = singles.tile([P, ET], F32, name="bq_pt", tag="bq_pt")
            nc.gpsimd.dma_start(out=bq_pt, in_=bqd[:].rearrange("(t p) -> p t", p=P))
            bk_pt = singles.tile([P, ET], F32, name="bk_pt", tag="bk_pt")
            nc.gpsimd.dma_start(out=bk_pt, in_=bkd[:].rearrange("(t p) -> p t", p=P))
            # broadcast bias layouts [128, D] (e on free dim)
            bv_bc = singles.tile([P, D], F32, name="bv_bc", tag="bv_bc")
            nc.gpsimd.dma_start(out=bv_bc, in_=bcast_ap(bvd))
            rzt = singles.tile([P, SQT], F32, name="rzt", tag="rzt")

            if True:
                # Q phase (queries = first SQ cols of xT)
                qt_pool = tc.alloc_tile_pool(name="qt", bufs=ET, side="right")
                qt_tiles = [qt_pool.tile([P, SQ], F32R, name=f"qt{i}", tag="qt")
                            for i in range(ET)]
                for et in range(ET):
                    wq_col = wq_next[et] if et < 2 else load_wcol(wqT, et, "wq")
                    for sb in range(SQB):
                        pq = psum_mm.tile([P, SQW], F32, name="pq", tag="mm")
                        for d in range(DT):
                            nc.tensor.matmul(
                                pq,
                                lhsT=(wq_col[:, d, :]),
                                rhs=xt_slice(d, sb * SQW, SQW),
                                start=(d == 0), stop=(d == DT - 1),
                            )
                        nc.scalar.activation(
                            out=qt_tiles[et][:, sb * SQW:(sb + 1) * SQW],
                            in_=pq, func=AF.Identity,
                            bias=bq_pt[:, et:et + 1], scale=1.0,
                        )

            # xtb (key-tail half of xT) + first wk columns: prefetched during
            # the tail of the Q phase
            for t in range(DT):
                xtb = xt_pool.tile([P, S - SQ], F32R, name=f"xtb{t}", tag="xt")
                nc.sync.dma_start(out=xtb, in_=xr[t][:, SQ:S])
                xtb_tiles.append(xtb)
            wk_next = [load_wcol(wkT, 0, "wk"), load_wcol(wkT, 1, "wk")]

            # wv column for the V phase: pool opened (and first column
            # loaded) before the fly pool so its address range is virgin --
            # no release-dep gating -- and the data streams in during K
            wv_pool = tc.alloc_tile_pool(name="wvcol", bufs=1)

            def load_wv(eb):
                wv_col = wv_pool.tile([P, DT, NBW], F32R, name="wv", tag="wv")
                wvr = wvT[:, eb * NBW:(eb + 1) * NBW].rearrange(
                    "(t p) e -> t p e", p=P)
                for d in range(DT):
                    nc.sync.dma_start(out=wv_col[:, d, :], in_=wvr[d])
                return wv_col

            wv_first = load_wv(0)

            # K phase -> DRAM scratch
            with tc.tile_pool(name="fly", bufs=3) as fly_pool:
                for et in range(ET):
                    wk_col = wk_next[et] if et < 2 else load_wcol(wkT, et, "wk")
                    for sb in range(SKB):
                        pk = psum_mm.tile([P, SBW], F32, name="pk", tag="mm")
                        for d in range(DT):
                            nc.tensor.matmul(
                                pk,
                                lhsT=(wk_col[:, d, :]),
                                rhs=xt_slice(d, sb * SBW, SBW),
                                start=(d == 0), stop=(d == DT - 1),
                            )
                        ktf = fly_pool.tile([P, SBW], F32R, name="ktf", tag="fly")
                        nc.scalar.activation(
                            out=ktf, in_=pk, func=AF.Identity,
                            bias=bk_pt[:, et:et + 1], scale=1.0,
                        )
                        nc.gpsimd.dma_start(
                            out=ktd[et * P:(et + 1) * P, sb * SBW:(sb + 1) * SBW],
                            in_=ktf,
                        )
            # V phase
            if True:
                for eb in range(NB):
                    wv_col = wv_first if eb == 0 else load_wv(eb)
                    for s in range(SKT):
                        pv = psum_mm.tile([P, NBW], F32, name="pv", tag="mm")
                        for d in range(DT):
                            nc.tensor.matmul(
                                pv,
                                lhsT=xt_slice(d, s * P, P),
                                rhs=(wv_col[:, d, :]),
                                start=(d == 0), stop=(d == DT - 1),
                            )
                        nc.vector.scalar_tensor_tensor(
                            out=v_tiles[s][:, eb * NBW:(eb + 1) * NBW],
                            in0=pv, scalar=1.0,
                            in1=bv_bc[:, eb * NBW:(eb + 1) * NBW],
                            op0=ALU.mult, op1=ALU.add,
                        )

            wv_pool.release()
            wc_pool.release()

        # ---------------- scores + Z (KT streamed back) ----------------
        u_pool = tc.alloc_tile_pool(name="u", bufs=SKT * SQB)
        u_tiles = [[None] * SKT for _ in range(SQB)]
        with tc.tile_pool(name="ktcol", bufs=2, side="right") as kt_pool:
            pz = [psum_z.tile([1, SQW], F32, name=f"pz{q}", tag="z")
                  for q in range(SQB)]
            for sk in range(SKT):
                kt_col = kt_pool.tile([P, ET, P], F32R, name="ktc", tag="ktc")
                nc.sync.dma_start(
                    out=kt_col,
                    in_=ktd[:, sk * P:(sk + 1) * P].rearrange("(t p) s -> p t s", p=P),
                )
                for q in range(SQB):
                    ps = psum_mm.tile([P, SQW], F32, name="ps", tag="mm")
                    for e in range(ET):
                        nc.tensor.matmul(
                            ps,
                            lhsT=(kt_col[:, e, :]),
                            rhs=(qt_tiles[e][:, q * SQW:(q + 1) * SQW]),
                            start=(e == 0), stop=(e == ET - 1),
                        )
                    ut = u_pool.tile([P, SQW], F32R, name=f"u{q}_{sk}", tag="u")
                    nc.scalar.activation(out=ut, in_=ps, func=AF.Exp, scale=SCALE)
                    u_tiles[q][sk] = ut
                    nc.tensor.matmul(
                        pz[q], lhsT=(ones_col), rhs=(ut),
                        start=(sk == 0), stop=(sk == SKT - 1),
                    )
            # Z -> 1/Z transposed to per-partition layout
            for q in range(SQB):
                z_sb = kt_pool.tile([1, SQW], F32, name="z_sb", tag="z_sb", bufs=1)
                nc.scalar.copy(z_sb, pz[q])
                for j in range(SQW // P):
                    pt = psum_tr.tile([P, 1], F32, name="pt", tag="tr")
                    nc.tensor.transpose(
                        pt, z_sb[0:1, j * P:(j + 1) * P], ident[0:1, 0:1])
                    jj = q * (SQW // P) + j
                    nc.vector.reciprocal(out=rzt[:, jj:jj + 1], in_=pt)
        qt_pool.release()

        # ---------------- ctx phase ----------------
        ctx_pool = tc.alloc_tile_pool(name="ctx", bufs=ET, side="right")
        ctx_tiles = [ctx_pool.tile([P, SQ], F32R, name=f"ctx{i}", tag="ctx")
                     for i in range(ET)]
        # prefetch first wo column during ctx (16KB; 2nd column streams later)
        wo_pool = tc.alloc_tile_pool(name="wocol", bufs=1, side="right")
        bo_bc = wo_pool.tile([P, D], F32, name="bo_bc", tag="bo_bc")
        nc.gpsimd.dma_start(out=bo_bc, in_=bcast_ap(bod))

        def load_wo(fb):
            wo_col = wo_pool.tile([P, DT, NBW], F32R, name="wo", tag="wo")
            nc.scalar.dma_start(
                out=wo_col,
                in_=woT[:, fb * NBW:(fb + 1) * NBW].rearrange(
                    "(t p) f -> p t f", p=P),
            )
            return wo_col

        wo_first = load_wo(0)
        for e in range(ET):
            for q in range(SQB):
                pc = psum_mm.tile([P, SQW], F32, name="pc", tag="mm")
                for sk in range(SKT):
                    nc.tensor.matmul(
                        pc,
                        lhsT=(v_tiles[sk][:, e * P:(e + 1) * P]),
                        rhs=(u_tiles[q][sk]),
                        start=(sk == 0), stop=(sk == SKT - 1),
                    )
                nc.scalar.copy(ctx_tiles[e][:, q * SQW:(q + 1) * SQW], pc)
        u_pool.release()

        # ---------------- out projection ----------------
        with tc.tile_pool(name="ofly", bufs=3, side="right") as o_pool:
            for fb in range(NB):
                wo_col = wo_first if fb == 0 else load_wo(fb)
                for st in range(SQT):
                    po = psum_mm.tile([P, NBW], F32, name="po", tag="mm")
                    for e in range(ET):
                        nc.tensor.matmul(
                            po,
                            lhsT=(ctx_tiles[e][:, st * P:(st + 1) * P]),
                            rhs=(wo_col[:, e, :]),
                            start=(e == 0), stop=(e == ET - 1),
                        )
                    osb = o_pool.tile([P, NBW], F32, name="osb", tag="ofly")
                    nc.vector.scalar_tensor_tensor(
                        out=osb, in0=po, scalar=rzt[:, st:st + 1],
                        in1=bo_bc[:, fb * NBW:(fb + 1) * NBW],
                        op0=ALU.mult, op1=ALU.add,
                    )
                    nc.scalar.dma_start(
                        out=outd[st * P:(st + 1) * P, fb * NBW:(fb + 1) * NBW],
                        in_=osb,
                    )
        wo_pool.release()
        ctx_pool.release()
        v_pool.release()

    nc.compile()
    return nc


_NC_CACHE = {}


def _get_nc():
    if "nc" not in _NC_CACHE:
        _NC_CACHE["nc"] = build_nc()
    return _NC_CACHE["nc"]


def _round_f32r(a):
    """Round-to-nearest to fp32r precision (fp22 = s1e8m13), so the PE's
    read-truncation behaves like round-to-nearest overall."""
    u = np.ascontiguousarray(a, np.float32).view(np.uint32)
    u = ((u.astype(np.uint64) + 0x200) & 0xFFFFFC00).astype(np.uint32)
    return u.view(np.float32)


def make_in_maps(x, Wq, bq, Wk, bk, Wv, bv, Wo, bo):
    x = _round_f32r(np.asarray(x, dtype=np.float32))
    wqT = _round_f32r(np.asarray(Wq, np.float32).T)
    wkT = _round_f32r(np.asarray(Wk, np.float32).T)
    wvT = _round_f32r(np.asarray(Wv, np.float32).T)
    woT = _round_f32r(np.asarray(Wo, np.float32).T)
    bq = np.ascontiguousarray(np.asarray(bq, np.float32))
    bk = np.ascontiguousarray(np.asarray(bk, np.float32))
    bv = np.ascontiguousarray(np.asarray(bv, np.float32))
    bo = np.ascontiguousarray(np.asarray(bo, np.float32))

    in_maps = []
    for c in range(N_CORES):
        b, h = c // 2, c % 2
        xb = x[b]  # [S, D]
        mine = xb[h * SQ:(h + 1) * SQ]
        other = xb[(1 - h) * SQ:(2 - h) * SQ]
        xTc = np.ascontiguousarray(np.concatenate([mine, other], axis=0).T)
        in_maps.append({
            "xT": xTc, "wqT": wqT, "wkT": wkT, "wvT": wvT, "woT": woT,
            "bq": bq, "bk": bk, "bv": bv, "bo": bo,
        })
    return in_maps


def assemble(results):
    out = np.empty((B, S, D), np.float32)
    for c in range(N_CORES):
        b, h = c // 2, c % 2
        out[b, h * SQ:(h + 1) * SQ] = results[c]["out"]
    return out


def kernel(x, Wq, bq, Wk, bk, Wv, bv, Wo, bo, **kwargs):
    nc = _get_nc()
    in_maps = make_in_maps(x, Wq, bq, Wk, bk, Wv, bv, Wo, bo)
    res = bass_utils.run_bass_kernel_spmd(nc, in_maps, core_ids=list(range(N_CORES)))
    return assemble(res.results)



# revision 7
# speedup vs baseline: 1.7551x; 1.7551x over previous
"""Single-head attention (nn_MultiHeadAttention) Trainium2 Bass kernel, v2.

Full inputs: x [4, 2048, 1024], Wq/Wk/Wv/Wo [1024, 1024], biases [1024].
reference:  q = x @ Wq.T + bq ; k,v likewise
            scores = (q @ k.T) / sqrt(1024) ; attn = softmax(scores, -1)
            out = (attn @ v) @ Wo.T + bo

Sharding: 8 cores = 4 batches x 2 query-halves; each core owns 1024
queries and all 2048 keys of its batch.

Algebraic fusion: scores = x (Wq^T Wk) x^T + (bq Wk) x^T + per-query
consts (which cancel in softmax).  A = Wq^T Wk is precomputed on the
host, so the K projection (and its DRAM spill) disappears: scores
contract directly against the resident x tiles.  The per-key offset
o_k = x_k . (bq Wk) rides in through the exp's per-partition bias.

Per-core pipeline (all matmul operands bf16, fp32 PSUM accumulation):
  QA phase:  QAT[d',q] = A^T x_q^T       (d-outer: PE starts after the
                                          first 0.25MB of A and x land)
  scores:    u[k,q]    = exp((QAT^T x)^T * scale + o_k * scale)
             Z[q]      = sum_k u        (vector-engine accumulation,
                                         cross-partition via ones-matmul)
  V phase:   V[s,e]    = x^T Wv^T + bv
  ctx:       ctxT[e,q] = V^T u
  out:       out[q,f]  = (ctxT^T Wo^T) * (1/Z) + bo
"""

import numpy as np
from contextlib import ExitStack

import ml_dtypes

import concourse.bass as bass
import concourse.bacc as bacc
import concourse.mybir as mybir
import concourse.tile as tile
from concourse import bass_utils

F32 = mybir.dt.float32
F32R = mybir.dt.float32r
BF16 = mybir.dt.bfloat16
AF = mybir.ActivationFunctionType
ALU = mybir.AluOpType

B, S, D = 4, 2048, 1024
SQ = S // 2  # queries per core
N_CORES = 8
SCALE = 1.0 / float(np.sqrt(D))

# matmul operand dtypes (PSUM accumulation is always fp32)
G1DT = BF16   # x, A, qa, wv  (QA / scores / V matmuls)
G2DT = BF16   # v, u          (ctx matmuls)
G3DT = BF16   # ctx, wo       (out-projection matmuls)


def build_nc():
    P = 128
    DT = D // P          # contraction tiles (8)
    ET = D // P          # output-dim tiles (8)
    SQW = 512            # query free-dim block
    SQB = SQ // SQW      # (2)
    SQT = SQ // P        # query tiles (8)
    SKT = S // P         # key tiles (16)
    NBW = 512            # free-dim block over D for V/out phases
    NB = D // NBW        # (2)

    nc = bacc.Bacc("TRN2", target_bir_lowering=False, debug=False)

    xT = nc.dram_tensor("xT", [D, S], G1DT, kind="ExternalInput")
    aM = nc.dram_tensor("aM", [D, D], G1DT, kind="ExternalInput")
    wvT = nc.dram_tensor("wvT", [D, D], G1DT, kind="ExternalInput")
    woT = nc.dram_tensor("woT", [D, D], G3DT, kind="ExternalInput")
    bvd = nc.dram_tensor("bv", [D], F32, kind="ExternalInput")
    bod = nc.dram_tensor("bo", [D], F32, kind="ExternalInput")
    soffd = nc.dram_tensor("soff", [S], F32, kind="ExternalInput")
    outd = nc.dram_tensor("out", [SQ, D], F32, kind="ExternalOutput")

    def bcast_ap(handle):
        a = handle[:]
        return bass.AP(tensor=a.tensor, offset=a.offset, ap=[[0, P]] + list(a.ap))

    with tile.TileContext(nc) as tc, ExitStack() as top:
        psum = top.enter_context(tc.tile_pool(name="psum", bufs=8, space="PSUM"))
        dram = top.enter_context(tc.tile_pool(name="dram", bufs=1, space="DRAM"))
        singles = top.enter_context(tc.tile_pool(name="singles", bufs=1))
        zscr = dram.tile([SQ], F32, name="zscr", tag="zscr")

        # ---- right-side pools, reserved in release order (LIFO top last)
        v_pool = tc.alloc_tile_pool(name="v", bufs=SKT, side="right")
        v_tiles = [v_pool.tile([P, D], G2DT, name=f"v{i}", tag="v")
                   for i in range(SKT)]
        u_pool = tc.alloc_tile_pool(name="u", bufs=SKT * SQB, side="right")
        u_tiles = [[None] * SKT for _ in range(SQB)]
        zacc_pool = tc.alloc_tile_pool(name="zacc", bufs=SQB, side="right")
        wv_pool = tc.alloc_tile_pool(name="wv", bufs=1, side="right")

        # ---- left-side: xt under qa under a_row (released in reverse)
        xt_pool = tc.alloc_tile_pool(name="xt", bufs=2 * DT)
        xr = xT[:].rearrange("(t p) s -> t p s", p=P)
        xta_tiles = []
        xtb_tiles = []
        for t in range(DT):
            xta = xt_pool.tile([P, SQ], G1DT, name=f"xta{t}", tag="xt")
            nc.sync.dma_start(out=xta, in_=xr[t][:, 0:SQ])
            xta_tiles.append(xta)

        def xt_slice(d, lo, width):
            if lo < SQ:
                return xta_tiles[d][:, lo:lo + width]
            return xtb_tiles[d][:, lo - SQ:lo - SQ + width]

        qa_pool = tc.alloc_tile_pool(name="qa", bufs=ET)
        qa_tiles = [qa_pool.tile([P, SQ], G1DT, name=f"qa{i}", tag="qa")
                    for i in range(ET)]
        a_pool = tc.alloc_tile_pool(name="arow", bufs=DT)
        a_rows = []
        for d in range(DT):
            ar = a_pool.tile([P, D], G1DT, name=f"ar{d}", tag="ar")
            nc.gpsimd.dma_start(out=ar, in_=aM[d * P:(d + 1) * P, :])
            a_rows.append(ar)

        # key-half tail of x + wv: streamed in behind the QA phase
        for t in range(DT):
            xtb = xt_pool.tile([P, S - SQ], G1DT, name=f"xtb{t}", tag="xt")
            nc.sync.dma_start(out=xtb, in_=xr[t][:, SQ:S])
            xtb_tiles.append(xtb)
        wv_full = wv_pool.tile([P, DT, D], G1DT, name="wv", tag="wv")
        wvr = wvT[:].rearrange("(t p) e -> t p e", p=P)
        for d in range(DT):
            nc.gpsimd.dma_start(out=wv_full[:, d, :], in_=wvr[d])

        # constants / biases (emitted after the start-critical loads)
        ones_f32 = singles.tile([P, 1], F32, name="ones_f32", tag="ones_f32")
        nc.vector.memset(ones_f32, 1.0)
        ones_col = singles.tile([P, 1], F32R, name="ones_col", tag="ones_col")
        nc.scalar.activation(out=ones_col, in_=ones_f32, func=AF.Copy)
        soff_pt = singles.tile([P, SKT], F32, name="soff_pt", tag="soff_pt")
        nc.gpsimd.dma_start(out=soff_pt, in_=soffd[:].rearrange("(t p) -> p t", p=P))
        bv_bc = singles.tile([P, D], F32, name="bv_bc", tag="bv_bc")
        nc.gpsimd.dma_start(out=bv_bc, in_=bcast_ap(bvd))
        rzt = singles.tile([P, SQT], F32, name="rzt", tag="rzt")
        zt = singles.tile([P, SQT], F32, name="zt", tag="zt")
        z_sb = singles.tile([1, SQ], F32, name="z_sb", tag="z_sb")

        # ---------------- QA phase (d-outer for fast start) ----------------
        for sb in range(SQB):
            pq = [psum.tile([P, SQW], F32, name="mm", tag="mm") for _ in range(ET)]
            for d in range(DT):
                for et in range(ET):
                    nc.tensor.matmul(
                        pq[et],
                        lhsT=a_rows[d][:, et * P:(et + 1) * P],
                        rhs=xta_tiles[d][:, sb * SQW:(sb + 1) * SQW],
                        start=(d == 0), stop=(d == DT - 1),
                    )
            for et in range(ET):
                nc.scalar.activation(
                    out=qa_tiles[et][:, sb * SQW:(sb + 1) * SQW],
                    in_=pq[et], func=AF.Copy,
                )
        a_pool.release()

        # ---------------- scores + Z ----------------
        for sk in range(SKT):
            for q in range(SQB):
                ps = psum.tile([P, SQW], F32, name="mm", tag="mm")
                for e in range(ET):
                    nc.tensor.matmul(
                        ps,
                        lhsT=xt_slice(e, sk * P, P),
                        rhs=qa_tiles[e][:, q * SQW:(q + 1) * SQW],
                        start=(e == 0), stop=(e == ET - 1),
                    )
                ut = u_pool.tile([P, SQW], G2DT, name=f"u{q}_{sk}", tag="u")
                nc.scalar.activation(
                    out=ut, in_=ps, func=AF.Exp,
                    bias=soff_pt[:, sk:sk + 1], scale=SCALE,
                )
                u_tiles[q][sk] = ut
                if sk == 0:
                    za = zacc_pool.tile([P, SQW], F32R, name=f"za{q}", tag="za")
                    nc.vector.tensor_copy(za, ut)
                    if q == 0:
                        zacc = [za]
                    else:
                        zacc.append(za)
                else:
                    nc.vector.tensor_tensor(
                        out=zacc[q], in0=zacc[q], in1=ut, op=ALU.add)

        # Z -> 1/Z in [q_p, st] layout (DRAM round-trip transpose)
        for q in range(SQB):
            pz = psum.tile([1, SQW], F32, name="mm", tag="mm")
            nc.tensor.matmul(pz, lhsT=ones_col, rhs=zacc[q], start=True, stop=True)
            nc.scalar.copy(z_sb[0:1, q * SQW:(q + 1) * SQW], pz)
        nc.gpsimd.dma_start(out=zscr, in_=z_sb[0:1, :])
        nc.gpsimd.dma_start(out=zt, in_=zscr[:].rearrange("(t p) -> p t", p=P))
        nc.vector.reciprocal(out=rzt, in_=zt)

        # ---------------- V phase ----------------
        for s in range(SKT):
            for eb in range(NB):
                pv = psum.tile([P, NBW], F32, name="mm", tag="mm")
                for d in range(DT):
                    nc.tensor.matmul(
                        pv,
                        lhsT=xt_slice(d, s * P, P),
                        rhs=wv_full[:, d, eb * NBW:(eb + 1) * NBW],
                        start=(d == 0), stop=(d == DT - 1),
                    )
                nc.vector.scalar_tensor_tensor(
                    out=v_tiles[s][:, eb * NBW:(eb + 1) * NBW],
                    in0=pv, scalar=1.0,
                    in1=bv_bc[:, eb * NBW:(eb + 1) * NBW],
                    op0=ALU.mult, op1=ALU.add,
                )
        wv_pool.release()
        zacc_pool.release()
        qa_pool.release()
        xt_pool.release()

        # ---------------- ctx phase (wo streams in behind it) ----------------
        ctx_pool = tc.alloc_tile_pool(name="ctx", bufs=ET)
        ctx_tiles = [ctx_pool.tile([P, SQ], G3DT, name=f"ctx{i}", tag="ctx")
                     for i in range(ET)]
        wo_pool = tc.alloc_tile_pool(name="wo", bufs=1)
        wo_full = wo_pool.tile([P, ET, D], G3DT, name="wo", tag="wo")
        wor = woT[:].rearrange("(t p) f -> t p f", p=P)
        for e in range(ET):
            nc.sync.dma_start(out=wo_full[:, e, :], in_=wor[e])
        bo_bc = singles.tile([P, D], F32, name="bo_bc", tag="bo_bc")
        nc.gpsimd.dma_start(out=bo_bc, in_=bcast_ap(bod))

        for q in range(SQB):
            for e in range(ET):
                pc = psum.tile([P, SQW], F32, name="mm", tag="mm")
                for sk in range(SKT):
                    nc.tensor.matmul(
                        pc,
                        lhsT=v_tiles[sk][:, e * P:(e + 1) * P],
                        rhs=u_tiles[q][sk],
                        start=(sk == 0), stop=(sk == SKT - 1),
                    )
                nc.scalar.copy(ctx_tiles[e][:, q * SQW:(q + 1) * SQW], pc)
        u_pool.release()
        v_pool.release()

        # ---------------- out projection ----------------
        with tc.tile_pool(name="ofly", bufs=3) as o_pool:
            for st in range(SQT):
                for fb in range(NB):
                    po = psum.tile([P, NBW], F32, name="mm", tag="mm")
                    for e in range(ET):
                        nc.tensor.matmul(
                            po,
                            lhsT=ctx_tiles[e][:, st * P:(st + 1) * P],
                            rhs=wo_full[:, e, fb * NBW:(fb + 1) * NBW],
                            start=(e == 0), stop=(e == ET - 1),
                        )
                    osb = o_pool.tile([P, NBW], F32, name="osb", tag="ofly")
                    nc.vector.scalar_tensor_tensor(
                        out=osb, in0=po, scalar=rzt[:, st:st + 1],
                        in1=bo_bc[:, fb * NBW:(fb + 1) * NBW],
                        op0=ALU.mult, op1=ALU.add,
                    )
                    nc.scalar.dma_start(
                        out=outd[st * P:(st + 1) * P, fb * NBW:(fb + 1) * NBW],
                        in_=osb,
                    )
        wo_pool.release()
        ctx_pool.release()

    nc.compile()
    return nc


_NC_CACHE = {}


def _get_nc():
    if "nc" not in _NC_CACHE:
        _NC_CACHE["nc"] = build_nc()
    return _NC_CACHE["nc"]


def _round_f32r(a):
    """Round-to-nearest to fp32r precision (fp22 = s1e8m13)."""
    u = np.ascontiguousarray(a, np.float32).view(np.uint32)
    u = ((u.astype(np.uint64) + 0x200) & 0xFFFFFC00).astype(np.uint32)
    return u.view(np.float32)


def _cast(a, dt):
    a = np.ascontiguousarray(np.asarray(a, np.float32))
    if dt == BF16:
        return a.astype(ml_dtypes.bfloat16)
    if dt == F32R:
        return _round_f32r(a)
    return a


def make_in_maps(x, Wq, bq, Wk, bk, Wv, bv, Wo, bo):
    x = np.asarray(x, np.float32)
    Wq = np.asarray(Wq, np.float32)
    Wk = np.asarray(Wk, np.float32)
    # A = Wq^T Wk so scores = x A x^T (+ per-key offset from bq, see header)
    aM = _cast(Wq.T @ Wk, G1DT)
    wvT = _cast(np.asarray(Wv, np.float32).T, G1DT)
    woT = _cast(np.asarray(Wo, np.float32).T, G3DT)
    bv = np.ascontiguousarray(np.asarray(bv, np.float32))
    bo = np.ascontiguousarray(np.asarray(bo, np.float32))
    ck = np.asarray(bq, np.float32) @ Wk  # [d]

    in_maps = []
    for c in range(N_CORES):
        b, h = c // 2, c % 2
        xb = x[b]  # [S, D]
        mine = xb[h * SQ:(h + 1) * SQ]
        other = xb[(1 - h) * SQ:(2 - h) * SQ]
        perm = np.concatenate([mine, other], axis=0)  # [S, D], own queries first
        xTc = _cast(perm.T, G1DT)
        soff = np.ascontiguousarray((perm @ ck) * np.float32(SCALE))
        in_maps.append({
            "xT": xTc, "aM": aM, "wvT": wvT, "woT": woT,
            "bv": bv, "bo": bo, "soff": soff,
        })
    return in_maps


def assemble(results):
    out = np.empty((B, S, D), np.float32)
    for c in range(N_CORES):
        b, h = c // 2, c % 2
        out[b, h * SQ:(h + 1) * SQ] = results[c]["out"]
    return out


def kernel(x, Wq, bq, Wk, bk, Wv, bv, Wo, bo, **kwargs):
    nc = _get_nc()
    in_maps = make_in_maps(x, Wq, bq, Wk, bk, Wv, bv, Wo, bo)
    res = bass_utils.run_bass_kernel_spmd(nc, in_maps, core_ids=list(range(N_CORES)))
    return assemble(res.results)


# revision 12
# speedup vs baseline: 1.7638x; 1.0050x over previous
"""Single-head attention (nn_MultiHeadAttention) Trainium2 Bass kernel, v2.

Full inputs: x [4, 2048, 1024], Wq/Wk/Wv/Wo [1024, 1024], biases [1024].
reference:  q = x @ Wq.T + bq ; k,v likewise
            scores = (q @ k.T) / sqrt(1024) ; attn = softmax(scores, -1)
            out = (attn @ v) @ Wo.T + bo

Sharding: 8 cores = 4 batches x 2 query-halves; each core owns 1024
queries and all 2048 keys of its batch.

Algebraic fusion: scores = x (Wq^T Wk) x^T + (bq Wk) x^T + per-query
consts (which cancel in softmax).  A = Wq^T Wk is precomputed on the
host, so the K projection (and its DRAM spill) disappears: scores
contract directly against the resident x tiles.  The per-key offset
o_k = x_k . (bq Wk) rides in through the exp's per-partition bias.

Per-core pipeline (all matmul operands bf16, fp32 PSUM accumulation):
  QA phase:  QAT[d',q] = A^T x_q^T       (d-outer: PE starts after the
                                          first 0.25MB of A and x land)
  scores:    u[k,q]    = exp((QAT^T x)^T * scale + o_k * scale)
             Z[q]      = sum_k u        (vector-engine accumulation,
                                         cross-partition via ones-matmul)
  V phase:   V[s,e]    = x^T Wv^T + bv
  ctx:       ctxT[e,q] = V^T u
  out:       out[q,f]  = (ctxT^T Wo^T) * (1/Z) + bo
"""

import numpy as np
from contextlib import ExitStack

import ml_dtypes

import concourse.bass as bass
import concourse.bacc as bacc
import concourse.mybir as mybir
import concourse.tile as tile
from concourse import bass_utils

F32 = mybir.dt.float32
F32R = mybir.dt.float32r
BF16 = mybir.dt.bfloat16
AF = mybir.ActivationFunctionType
ALU = mybir.AluOpType

B, S, D = 4, 2048, 1024
SQ = S // 2  # queries per core
N_CORES = 8
SCALE = 1.0 / float(np.sqrt(D))

# matmul operand dtypes (PSUM accumulation is always fp32)
G1DT = BF16   # x, A, qa, wv  (QA / scores / V matmuls)
G2DT = BF16   # v, u          (ctx matmuls)
G3DT = BF16   # ctx, wo       (out-projection matmuls)


def build_nc():
    P = 128
    DT = D // P          # contraction tiles (8)
    ET = D // P          # output-dim tiles (8)
    SQW = 512            # query free-dim block
    SQB = SQ // SQW      # (2)
    SQT = SQ // P        # query tiles (8)
    SKT = S // P         # key tiles (16)
    NBW = 512            # free-dim block over D for V/out phases
    NB = D // NBW        # (2)

    nc = bacc.Bacc("TRN2", target_bir_lowering=False, debug=False)

    xT = nc.dram_tensor("xT", [D, S], G1DT, kind="ExternalInput")
    aM = nc.dram_tensor("aM", [D, D], G1DT, kind="ExternalInput")
    wvT = nc.dram_tensor("wvT", [D, D], G1DT, kind="ExternalInput")
    woT = nc.dram_tensor("woT", [D, D], G3DT, kind="ExternalInput")
    bvd = nc.dram_tensor("bv", [D], F32, kind="ExternalInput")
    bod = nc.dram_tensor("bo", [D], F32, kind="ExternalInput")
    soffd = nc.dram_tensor("soff", [S], F32, kind="ExternalInput")
    outd = nc.dram_tensor("out", [SQ, D], F32, kind="ExternalOutput")

    def bcast_ap(handle):
        a = handle[:]
        return bass.AP(tensor=a.tensor, offset=a.offset, ap=[[0, P]] + list(a.ap))

    with tile.TileContext(nc) as tc, ExitStack() as top:
        psum = top.enter_context(tc.tile_pool(name="psum", bufs=8, space="PSUM"))
        dram = top.enter_context(tc.tile_pool(name="dram", bufs=1, space="DRAM"))
        singles = top.enter_context(tc.tile_pool(name="singles", bufs=1))
        zscr = dram.tile([SQ], F32, name="zscr", tag="zscr")

        # ---- right-side pools, reserved in release order (LIFO top last)
        v_pool = tc.alloc_tile_pool(name="v", bufs=SKT, side="right")
        v_tiles = [v_pool.tile([P, D], G2DT, name=f"v{i}", tag="v")
                   for i in range(SKT)]
        u_pool = tc.alloc_tile_pool(name="u", bufs=SKT * SQB, side="right")
        u_tiles = [[None] * SKT for _ in range(SQB)]
        zacc_pool = tc.alloc_tile_pool(name="zacc", bufs=SQB, side="right")
        wv_pool = tc.alloc_tile_pool(name="wv", bufs=1, side="right")

        # ---- left-side: xt under qa under a_row (released in reverse)
        xt_pool = tc.alloc_tile_pool(name="xt", bufs=2 * DT)
        xr = xT[:].rearrange("(t p) s -> t p s", p=P)
        qa_pool = tc.alloc_tile_pool(name="qa", bufs=ET)
        qa_tiles = [qa_pool.tile([P, SQ], G1DT, name=f"qa{i}", tag="qa")
                    for i in range(ET)]
        a_pool = tc.alloc_tile_pool(name="arow", bufs=DT)

        # QA-phase inputs, split into half-tile chunks round-robined over the
        # three DMA-capable engine queues in consumption order: the d-loop of
        # sb=0 needs a_row[d] (both halves) + xta[d][:, 0:512]; the sb=1 pass
        # then needs the xta second halves.
        QE = [nc.sync, nc.gpsimd, nc.scalar]
        H = SQ // 2
        xta_tiles = []
        xtb_tiles = []
        a_rows = []
        rr = 0
        for d in range(DT):
            ar = a_pool.tile([P, D], G1DT, name=f"ar{d}", tag="ar")
            xta = xt_pool.tile([P, SQ], G1DT, name=f"xta{d}", tag="xt")
            for h in range(2):
                QE[rr % 3].dma_start(
                    out=ar[:, h * H:(h + 1) * H],
                    in_=aM[d * P:(d + 1) * P, h * H:(h + 1) * H])
                rr += 1
            QE[rr % 3].dma_start(out=xta[:, 0:H], in_=xr[d][:, 0:H])
            rr += 1
            a_rows.append(ar)
            xta_tiles.append(xta)
        for d in range(DT):
            QE[rr % 3].dma_start(out=xta_tiles[d][:, H:SQ], in_=xr[d][:, H:SQ])
            rr += 1

        def xt_slice(d, lo, width):
            if lo < SQ:
                return xta_tiles[d][:, lo:lo + width]
            return xtb_tiles[d][:, lo - SQ:lo - SQ + width]

        # key-half tail of x (needed from the scores phase on)
        for t in range(DT):
            xtb = xt_pool.tile([P, S - SQ], G1DT, name=f"xtb{t}", tag="xt")
            QE[rr % 3].dma_start(out=xtb, in_=xr[t][:, SQ:S])
            rr += 1
            xtb_tiles.append(xtb)
        wv_full = wv_pool.tile([P, DT, D], G1DT, name="wv", tag="wv")
        wvr = wvT[:].rearrange("(t p) e -> t p e", p=P)
        for d in range(DT):
            nc.gpsimd.dma_start(out=wv_full[:, d, :], in_=wvr[d])

        # constants / biases (emitted after the start-critical loads)
        ones_f32 = singles.tile([P, 1], F32, name="ones_f32", tag="ones_f32")
        nc.vector.memset(ones_f32, 1.0)
        ones_col = singles.tile([P, 1], F32R, name="ones_col", tag="ones_col")
        nc.scalar.activation(out=ones_col, in_=ones_f32, func=AF.Copy)
        soff_pt = singles.tile([P, SKT], F32, name="soff_pt", tag="soff_pt")
        nc.gpsimd.dma_start(out=soff_pt, in_=soffd[:].rearrange("(t p) -> p t", p=P))
        bv_bc = singles.tile([P, D], F32, name="bv_bc", tag="bv_bc")
        nc.gpsimd.dma_start(out=bv_bc, in_=bcast_ap(bvd))
        rzt = singles.tile([P, SQT], F32, name="rzt", tag="rzt")
        zt = singles.tile([P, SQT], F32, name="zt", tag="zt")
        z_sb = singles.tile([1, SQ], F32, name="z_sb", tag="z_sb")

        # ---------------- QA phase (d-outer for fast start) ----------------
        for sb in range(SQB):
            pq = [psum.tile([P, SQW], F32, name="mm", tag="mm") for _ in range(ET)]
            for d in range(DT):
                for et in range(ET):
                    nc.tensor.matmul(
                        pq[et],
                        lhsT=a_rows[d][:, et * P:(et + 1) * P],
                        rhs=xta_tiles[d][:, sb * SQW:(sb + 1) * SQW],
                        start=(d == 0), stop=(d == DT - 1),
                    )
            for et in range(ET):
                nc.scalar.activation(
                    out=qa_tiles[et][:, sb * SQW:(sb + 1) * SQW],
                    in_=pq[et], func=AF.Copy,
                )
        a_pool.release()

        # ---------------- scores + Z ----------------
        for sk in range(SKT):
            for q in range(SQB):
                ps = psum.tile([P, SQW], F32, name="mm", tag="mm")
                for e in range(ET):
                    nc.tensor.matmul(
                        ps,
                        lhsT=xt_slice(e, sk * P, P),
                        rhs=qa_tiles[e][:, q * SQW:(q + 1) * SQW],
                        start=(e == 0), stop=(e == ET - 1),
                    )
                ut = u_pool.tile([P, SQW], G2DT, name=f"u{q}_{sk}", tag="u")
                nc.scalar.activation(
                    out=ut, in_=ps, func=AF.Exp,
                    bias=soff_pt[:, sk:sk + 1], scale=SCALE,
                )
                u_tiles[q][sk] = ut
                if sk == 0:
                    za = zacc_pool.tile([P, SQW], F32R, name=f"za{q}", tag="za")
                    nc.vector.tensor_copy(za, ut)
                    if q == 0:
                        zacc = [za]
                    else:
                        zacc.append(za)
                else:
                    nc.vector.tensor_tensor(
                        out=zacc[q], in0=zacc[q], in1=ut, op=ALU.add)

        # ---------------- V phase ----------------
        for s in range(SKT):
            for eb in range(NB):
                pv = psum.tile([P, NBW], F32, name="mm", tag="mm")
                for d in range(DT):
                    nc.tensor.matmul(
                        pv,
                        lhsT=xt_slice(d, s * P, P),
                        rhs=wv_full[:, d, eb * NBW:(eb + 1) * NBW],
                        start=(d == 0), stop=(d == DT - 1),
                    )
                nc.vector.scalar_tensor_tensor(
                    out=v_tiles[s][:, eb * NBW:(eb + 1) * NBW],
                    in0=pv, scalar=1.0,
                    in1=bv_bc[:, eb * NBW:(eb + 1) * NBW],
                    op0=ALU.mult, op1=ALU.add,
                )
        # Z -> 1/Z in [q_p, st] layout (DRAM round-trip transpose); emitted
        # after the V matmuls so the z path never gates the PE stream
        for q in range(SQB):
            pz = psum.tile([1, SQW], F32, name="mm", tag="mm")
            nc.tensor.matmul(pz, lhsT=ones_col, rhs=zacc[q], start=True, stop=True)
            nc.scalar.copy(z_sb[0:1, q * SQW:(q + 1) * SQW], pz)
        nc.gpsimd.dma_start(out=zscr, in_=z_sb[0:1, :])
        nc.gpsimd.dma_start(out=zt, in_=zscr[:].rearrange("(t p) -> p t", p=P))
        nc.vector.reciprocal(out=rzt, in_=zt)

        wv_pool.release()
        zacc_pool.release()
        qa_pool.release()
        xt_pool.release()

        # ---------------- ctx phase (wo streams in behind it) ----------------
        ctx_pool = tc.alloc_tile_pool(name="ctx", bufs=ET)
        ctx_tiles = [ctx_pool.tile([P, SQ], G3DT, name=f"ctx{i}", tag="ctx")
                     for i in range(ET)]
        wo_pool = tc.alloc_tile_pool(name="wo", bufs=1)
        wo_full = wo_pool.tile([P, ET, D], G3DT, name="wo", tag="wo")
        wor = woT[:].rearrange("(t p) f -> t p f", p=P)
        for e in range(ET):
            nc.sync.dma_start(out=wo_full[:, e, :], in_=wor[e])
        bo_bc = singles.tile([P, D], F32, name="bo_bc", tag="bo_bc")
        nc.gpsimd.dma_start(out=bo_bc, in_=bcast_ap(bod))

        for q in range(SQB):
            for e in range(ET):
                pc = psum.tile([P, SQW], F32, name="mm", tag="mm")
                for sk in range(SKT):
                    nc.tensor.matmul(
                        pc,
                        lhsT=v_tiles[sk][:, e * P:(e + 1) * P],
                        rhs=u_tiles[q][sk],
                        start=(sk == 0), stop=(sk == SKT - 1),
                    )
                nc.scalar.copy(ctx_tiles[e][:, q * SQW:(q + 1) * SQW], pc)
        u_pool.release()
        v_pool.release()

        # ---------------- out projection ----------------
        with tc.tile_pool(name="ofly", bufs=3) as o_pool:
            for st in range(SQT):
                for fb in range(NB):
                    po = psum.tile([P, NBW], F32, name="mm", tag="mm")
                    for e in range(ET):
                        nc.tensor.matmul(
                            po,
                            lhsT=ctx_tiles[e][:, st * P:(st + 1) * P],
                            rhs=wo_full[:, e, fb * NBW:(fb + 1) * NBW],
                            start=(e == 0), stop=(e == ET - 1),
                        )
                    osb = o_pool.tile([P, NBW], F32, name="osb", tag="ofly")
                    nc.vector.scalar_tensor_tensor(
                        out=osb, in0=po, scalar=rzt[:, st:st + 1],
                        in1=bo_bc[:, fb * NBW:(fb + 1) * NBW],
                        op0=ALU.mult, op1=ALU.add,
                    )
                    nc.scalar.dma_start(
                        out=outd[st * P:(st + 1) * P, fb * NBW:(fb + 1) * NBW],
                        in_=osb,
                    )
        wo_pool.release()
        ctx_pool.release()

    nc.compile()
    return nc


_NC_CACHE = {}


def _get_nc():
    if "nc" not in _NC_CACHE:
        _NC_CACHE["nc"] = build_nc()
    return _NC_CACHE["nc"]


def _round_f32r(a):
    """Round-to-nearest to fp32r precision (fp22 = s1e8m13)."""
    u = np.ascontiguousarray(a, np.float32).view(np.uint32)
    u = ((u.astype(np.uint64) + 0x200) & 0xFFFFFC00).astype(np.uint32)
    return u.view(np.float32)


def _cast(a, dt):
    a = np.ascontiguousarray(np.asarray(a, np.float32))
    if dt == BF16:
        return a.astype(ml_dtypes.bfloat16)
    if dt == F32R:
        return _round_f32r(a)
    return a


def make_in_maps(x, Wq, bq, Wk, bk, Wv, bv, Wo, bo):
    x = np.asarray(x, np.float32)
    Wq = np.asarray(Wq, np.float32)
    Wk = np.asarray(Wk, np.float32)
    # A = Wq^T Wk so scores = x A x^T (+ per-key offset from bq, see header)
    aM = _cast(Wq.T @ Wk, G1DT)
    wvT = _cast(np.asarray(Wv, np.float32).T, G1DT)
    woT = _cast(np.asarray(Wo, np.float32).T, G3DT)
    bv = np.ascontiguousarray(np.asarray(bv, np.float32))
    bo = np.ascontiguousarray(np.asarray(bo, np.float32))
    ck = np.asarray(bq, np.float32) @ Wk  # [d]

    in_maps = []
    for c in range(N_CORES):
        b, h = c // 2, c % 2
        xb = x[b]  # [S, D]
        mine = xb[h * SQ:(h + 1) * SQ]
        other = xb[(1 - h) * SQ:(2 - h) * SQ]
        perm = np.concatenate([mine, other], axis=0)  # [S, D], own queries first
        xTc = _cast(perm.T, G1DT)
        soff = np.ascontiguousarray((perm @ ck) * np.float32(SCALE))
        in_maps.append({
            "xT": xTc, "aM": aM, "wvT": wvT, "woT": woT,
            "bv": bv, "bo": bo, "soff": soff,
        })
    return in_maps


def assemble(results):
    out = np.empty((B, S, D), np.float32)
    for c in range(N_CORES):
        b, h = c // 2, c % 2
        out[b, h * SQ:(h + 1) * SQ] = results[c]["out"]
    return out


def kernel(x, Wq, bq, Wk, bk, Wv, bv, Wo, bo, **kwargs):
    nc = _get_nc()
    in_maps = make_in_maps(x, Wq, bq, Wk, bk, Wv, bv, Wo, bo)
    res = bass_utils.run_bass_kernel_spmd(nc, in_maps, core_ids=list(range(N_CORES)))
    return assemble(res.results)
